# revision 9
# baseline (speedup 1.0000x reference)
"""Trainium2 Bass kernel for nn_CausalLM_36910948942461.

4-layer causal LM: D=1024, H=16 (GQA KVH=8, dk=64), SwiGLU FFN 2752, S=2048, B=2,
V=32000, tied embedding head, full logits out [2, 2048, 32000] fp32.

Sharding over 8 NeuronCores:
  - Attention: by (batch, head-pair). Core c owns q-heads {2c, 2c+1} + kv-head c for
    BOTH batches over the full 2048-token sequence (perfectly uniform SPMD).
  - Everything else (residual stream, norms, QKV/out projections, FFN): by token.
    Core c owns positions [256c, 256c+256) of each batch (512 tokens).
  - LM head: vocab-sharded (core c computes logit columns [4000c, 4000c+4000) for
    all 4096 tokens); host concatenates.
Per layer: AllGather of normalized activations (1 MB/rank) before QKV, AllToAll of
attention outputs (1 MB/rank) before the output projection.
Compute: bf16 matmuls with fp32 PSUM accumulation; fp32 residual stream.
RoPE: weight columns host-permuted to deinterleaved (r|i) layout; rotation on device
via a partition-swap permutation matmul + 3 elementwise ops with host cos/sin tables.
Softmax: scores computed transposed [key, query]; exp on ScalarE (no max subtraction:
logit range is tiny for this model); denominators via a ones-augmented V column.
"""

import numpy as np
import ml_dtypes

import concourse.bass as bass
import concourse.mybir as mybir
import concourse.tile as tile
from concourse import bacc, bass2jax

f32 = mybir.dt.float32
bf16 = mybir.dt.bfloat16
BF = ml_dtypes.bfloat16

# Model dims (hardcoded per problem spec)
N_CORES = 8
V, L, D, H, KVH, FD, S, B = 32000, 4, 1024, 16, 8, 2752, 2048, 2
DK = 64                      # head dim
DSUB = D // 128              # 8 partition tiles of the model dim
TB = S                       # tokens per batch (full sequence)
TC = S // N_CORES            # 256 tokens per (core, batch)
T = B * TC                   # 512 tokens owned per core
FDP = 2816                   # FD padded to 22*128
FM = FDP // 128              # 22
VSH = V // N_CORES           # 4000 vocab columns per core
VSHP = 4096                  # padded
EPS = 1e-6

_CACHE = {}


# ---------------------------------------------------------------- device program

def build_program():
    nc = bacc.Bacc("TRN2", target_bir_lowering=False, debug=False,
                   enable_asserts=True, num_devices=N_CORES)

    def din(name, shape, dt=bf16):
        return nc.dram_tensor(name, shape, dt, kind="ExternalInput").ap()

    # Inputs (per core). Weight layouts are host-pre-tiled for contiguous DMA.
    x0T = din("x0T", [128, DSUB, T], f32)          # embedded tokens, feature-major
    wq = din("wq", [L, 128, DSUB, 128])            # [l, p, o, m] q-head pair cols (perm, g1, /8)
    wk = din("wk", [L, 128, DSUB, DK])             # kv-head cols (perm, g1)
    wv = din("wv", [L, 128, DSUB, DK])             # (g1)
    wo = din("wo", [L, DSUB, 128, DSUB, 128])      # [l, m, p, kt, c]
    w1 = din("w1", [L, FM, 128, DSUB, 128])        # [l, m, p, o, c] (g2, padded)
    w2 = din("w2", [L, FM, 128, DSUB, 128])
    w3 = din("w3", [L, DSUB, 128, FM, 128])        # [l, m, p, ko, c] (padded)
    embH = din("embH", [128, DSUB, VSHP])          # head matrix slice (gpost folded)
    c1 = din("c1", [DK, TB], f32)                  # rope cos table (stacked x2)
    c2 = din("c2", [DK, TB], f32)                  # rope sin table (-sin | +sin)
    pswap = din("pswap", [DK, DK])                 # 32-half swap permutation
    ident = din("ident", [DK, DK])                 # 64x64 identity (for PE transpose)
    bq = din("bq", [L, 128], f32)
    bk = din("bk", [L, DK], f32)
    bvB = din("bvB", [L, 128, DK], f32)            # bv replicated over partitions
    bo = din("bo", [L, 128, DSUB], f32)
    b1 = din("b1", [L, 128, FM], f32)
    b2 = din("b2", [L, 128, FM], f32)
    b3 = din("b3", [L, 128, DSUB], f32)

    logits = nc.dram_tensor("logits", [N_CORES * T, VSHP], f32,
                            kind="ExternalOutput").ap()

    RG = [list(range(N_CORES))]

    with tile.TileContext(nc) as tc:
        # ---- persistent pools
        const = tc.alloc_tile_pool(name="const", bufs=1)
        xp = tc.alloc_tile_pool(name="xp", bufs=1)
        embp = tc.alloc_tile_pool(name="embp", bufs=1)
        dram = tc.alloc_tile_pool(name="dram", bufs=2, space="DRAM")

        c1_sb = const.tile([DK, TB], f32)
        c2_sb = const.tile([DK, TB], f32)
        pswap_sb = const.tile([DK, DK], bf16)
        ident64 = const.tile([DK, DK], bf16)
        ones_pcol = const.tile([128, 1], bf16)     # partition-sum lhsT
        ones_row = const.tile([1, 128], f32)       # K=1 broadcast lhsT
        nc.sync.dma_start(c1_sb[:], c1)
        nc.sync.dma_start(c2_sb[:], c2)
        nc.sync.dma_start(pswap_sb[:], pswap)
        nc.sync.dma_start(ident64[:], ident)
        eps_sb = const.tile([1, 1], f32)
        nc.any.memset(ones_pcol[:], 1.0)
        nc.any.memset(ones_row[:], 1.0)
        nc.any.memset(eps_sb[:], EPS)

        xT = xp.tile([128, DSUB, T], f32)
        nc.sync.dma_start(xT[:], x0T)

        def rmsnorm(pool, psum, src, name):
            """src [128, DSUB, T] f32 -> normalized bf16 (gains folded into weights)."""
            xsq = pool.tile([128, DSUB, T], bf16, tag="xsq", name=f"xsq_{name}", bufs=1)
            nc.scalar.activation(xsq[:], src[:], mybir.ActivationFunctionType.Square)
            ssq = psum.tile([1, T], f32, tag="ssq", name=f"ssq_{name}")
            for o in range(DSUB):
                nc.tensor.matmul(ssq[:], ones_pcol[:], xsq[:, o, :],
                                 start=(o == 0), stop=(o == DSUB - 1))
            srt = pool.tile([1, T], f32, tag="srt", name=f"srt_{name}")
            nc.scalar.activation(srt[:], ssq[:], mybir.ActivationFunctionType.Sqrt,
                                 scale=1.0 / D, bias=eps_sb[:])
            rs = pool.tile([1, T], f32, tag="rs", name=f"rs_{name}")
            nc.vector.reciprocal(rs[:], srt[:])
            rsB = psum.tile([128, T], f32, tag="rsB", name=f"rsB_{name}")
            nc.tensor.matmul(rsB[:], ones_row[:], rs[:], start=True, stop=True)
            out = pool.tile([128, DSUB, T], bf16, tag="hT", name=f"hT_{name}", bufs=1)
            nc.vector.tensor_tensor(
                out[:], src[:], rsB[:, None, :].to_broadcast((128, DSUB, T)),
                mybir.AluOpType.mult)
            return out

        hT_all_view = None
        for l in range(L):
            # ---------------- rmsnorm 1 + AllGather of normalized activations
            with tc.tile_pool(name="rmsp", bufs=1) as rp, \
                 tc.tile_pool(name="rmps", bufs=1, space="PSUM") as rps:
                hT = rmsnorm(rp, rps, xT, f"r1_{l}")
                ag_in = dram.tile([D, T], bf16, tag="ag_in")
                nc.sync.dma_start(ag_in.rearrange("(o p) t -> p o t", p=128), hT[:])
                hT_all = dram.tile([N_CORES * D, T], bf16, tag="hT_all",
                                   addr_space="Shared")
                nc.gpsimd.collective_compute(
                    "AllGather", mybir.AluOpType.bypass, replica_groups=RG,
                    ins=[ag_in[:]], outs=[hT_all[:]])
                hT_all_view = hT_all.rearrange("(r d) t -> r d t", r=N_CORES)

            # layer-scope pools for qkv + attention
            with tc.tile_pool(name="wqkv", bufs=2) as wp, \
                 tc.tile_pool(name="htb", bufs=2) as hp, \
                 tc.tile_pool(name="qkv", bufs=2) as qp, \
                 tc.tile_pool(name="attn", bufs=3) as ap, \
                 tc.tile_pool(name="pp", bufs=2, space="PSUM") as pp, \
                 tc.tile_pool(name="scp", bufs=1, space="PSUM") as scp, \
                 tc.tile_pool(name="op0", bufs=2, space="PSUM") as opp0, \
                 tc.tile_pool(name="op1", bufs=2, space="PSUM") as opp1, \
                 tc.tile_pool(name="a2ap", bufs=1, space="DRAM") as a2ap:

                wq_sb = wp.tile([128, DSUB, 128], bf16, tag="wq")
                wk_sb = wp.tile([128, DSUB, DK], bf16, tag="wk")
                wv_sb = wp.tile([128, DSUB, DK], bf16, tag="wv")
                nc.sync.dma_start(wq_sb[:], wq[l])
                nc.sync.dma_start(wk_sb[:], wk[l])
                nc.sync.dma_start(wv_sb[:], wv[l])
                bq_sb = wp.tile([128, 1], f32, tag="bq")
                bk_sb = wp.tile([DK, 1], f32, tag="bk")
                bvB_sb = wp.tile([128, DK], f32, tag="bvB")
                nc.sync.dma_start(bq_sb[:], bq[l][:, None])
                nc.sync.dma_start(bk_sb[:], bk[l][:, None])
                nc.sync.dma_start(bvB_sb[:], bvB[l])

                a2a_in = a2ap.tile([N_CORES, B, 2, DK, TC], bf16, tag="a2a_in")
                a2a_out = a2ap.tile([N_CORES, B, 2, DK, TC], bf16, tag="a2a_out")

                for u in range(B):
                    # -------- q/k/v projections over this batch's 2048 tokens
                    qT = qp.tile([DK, 2, TB], bf16, tag="qT", bufs=1)
                    kT = qp.tile([DK, TB], bf16, tag="kT", bufs=1)
                    v_aug = qp.tile([128, TB // 128, DK + 1], bf16, tag="vaug")
                    nc.any.memset(v_aug[:, :, DK:DK + 1], 1.0)
                    for n in range(4):  # 512-token chunks of this batch
                        htb = hp.tile([128, DSUB, 512], bf16, tag="htb")
                        for rr in range(2):
                            nc.sync.dma_start(
                                htb[:, :, 256 * rr:256 * (rr + 1)],
                                hT_all_view[2 * n + rr, :, TC * u:TC * (u + 1)]
                                .rearrange("(o p) t -> p o t", p=128))
                        cs = slice(512 * n, 512 * (n + 1))
                        q_ps = pp.tile([128, 512], f32, tag="pp")
                        for o in range(DSUB):
                            nc.tensor.matmul(q_ps[:], wq_sb[:, o, :], htb[:, o, :],
                                             start=(o == 0), stop=(o == DSUB - 1))
                        nc.vector.tensor_scalar_add(qT[:, 0, cs], q_ps[0:DK],
                                                    bq_sb[0:DK])
                        nc.vector.tensor_scalar_add(qT[:, 1, cs], q_ps[DK:128],
                                                    bq_sb[DK:128])
                        k_ps = pp.tile([DK, 512], f32, tag="pp")
                        for o in range(DSUB):
                            nc.tensor.matmul(k_ps[:], wk_sb[:, o, :], htb[:, o, :],
                                             start=(o == 0), stop=(o == DSUB - 1))
                        nc.vector.tensor_scalar_add(kT[:, cs], k_ps[:], bk_sb[:])
                        vT_ps = pp.tile([DK, 512], f32, tag="pp")
                        for o in range(DSUB):
                            nc.tensor.matmul(vT_ps[:], wv_sb[:, o, :], htb[:, o, :],
                                             start=(o == 0), stop=(o == DSUB - 1))
                        vT_sb = hp.tile([DK, 512], bf16, tag="vT")
                        nc.scalar.copy(vT_sb[:], vT_ps[:])
                        for sblk in range(4):
                            vtr = pp.tile([128, DK], bf16, tag="pp")
                            nc.tensor.transpose(vtr[:], vT_sb[:, 128 * sblk:128 * (sblk + 1)],
                                                ident64[:])
                            nc.vector.tensor_tensor(
                                v_aug[:, 4 * n + sblk, 0:DK], vtr[:], bvB_sb[:],
                                mybir.AluOpType.add)

                    # -------- RoPE (q and k)
                    qrT = qp.tile([DK, 2, TB], bf16, tag="qrT")
                    q_flat = qT.rearrange("p a t -> p (a t)")
                    qr_flat = qrT.rearrange("p a t -> p (a t)")
                    for c8 in range(8):
                        fs = slice(512 * c8, 512 * (c8 + 1))
                        ts_ = slice(512 * (c8 % 4), 512 * (c8 % 4 + 1))
                        sw_ps = pp.tile([DK, 512], f32, tag="pp")
                        nc.tensor.matmul(sw_ps[:], pswap_sb[:], q_flat[:, fs],
                                         start=True, stop=True)
                        tmp = hp.tile([DK, 512], bf16, tag="ropetmp")
                        nc.vector.tensor_tensor(tmp[:], sw_ps[:], c2_sb[:, ts_],
                                                mybir.AluOpType.mult)
                        nc.vector.tensor_tensor(qr_flat[:, fs], q_flat[:, fs],
                                                c1_sb[:, ts_], mybir.AluOpType.mult)
                        nc.vector.tensor_tensor(qr_flat[:, fs], qr_flat[:, fs],
                                                tmp[:], mybir.AluOpType.add)
                    krT = qp.tile([DK, TB], bf16, tag="krT")
                    for c4 in range(4):
                        fs = slice(512 * c4, 512 * (c4 + 1))
                        sw_ps = pp.tile([DK, 512], f32, tag="pp")
                        nc.tensor.matmul(sw_ps[:], pswap_sb[:], kT[:, fs],
                                         start=True, stop=True)
                        tmp = hp.tile([DK, 512], bf16, tag="ropetmp")
                        nc.vector.tensor_tensor(tmp[:], sw_ps[:], c2_sb[:, fs],
                                                mybir.AluOpType.mult)
                        nc.vector.tensor_tensor(krT[:, fs], kT[:, fs],
                                                c1_sb[:, fs], mybir.AluOpType.mult)
                        nc.vector.tensor_tensor(krT[:, fs], krT[:, fs],
                                                tmp[:], mybir.AluOpType.add)

                    # -------- attention: scoresT -> exp -> (mask) -> oT accumulate
                    opps = [opp0, opp1]
                    for qs in range(4):
                        o_ps = [opps[qh].tile([128, 512], f32, tag=f"ops{qh}",
                                              name=f"ops{qh}_{l}_{u}_{qs}")
                                for qh in range(2)]
                        nj = 4 * qs + 4
                        for j in range(nj):
                            doff = max(0, 128 * (j - 4 * qs))
                            tq = 512 - doff
                            sc_ps = scp.tile([128, 2, 512], f32, tag="sc",
                                             name=f"sc_{l}_{u}_{qs}_{j}")
                            for qh in range(2):
                                nc.tensor.matmul(
                                    sc_ps[:, qh, 0:tq],
                                    krT[:, 128 * j:128 * (j + 1)],
                                    qrT[:, qh, 512 * qs + doff:512 * (qs + 1)],
                                    start=True, stop=True)
                            ex = ap.tile([128, 2, 512], bf16, tag="exp",
                                         name=f"ex_{l}_{u}_{qs}_{j}")
                            nc.scalar.activation(ex[:, :, 0:tq], sc_ps[:, :, 0:tq],
                                                 mybir.ActivationFunctionType.Exp)
                            if j >= 4 * qs:
                                for qh in range(2):
                                    nc.gpsimd.affine_select(
                                        ex[:, qh, 0:128], ex[:, qh, 0:128],
                                        pattern=[[1, 128]],
                                        compare_op=mybir.AluOpType.is_ge,
                                        fill=0.0, base=0, channel_multiplier=-1)
                            for qh in range(2):
                                nc.tensor.matmul(
                                    o_ps[qh][0:DK + 1, doff:512],
                                    v_aug[:, j, :], ex[:, qh, 0:tq],
                                    start=(j == 0), stop=(j == nj - 1))
                        for qh in range(2):
                            recip = ap.tile([1, 512], f32, tag="recip")
                            nc.vector.reciprocal(recip[:], o_ps[qh][DK:DK + 1, :])
                            rb = ap.tile([DK, 512], f32, tag="rb")
                            nc.gpsimd.partition_broadcast(rb[:], recip[:])
                            o_sb = ap.tile([DK, 512], bf16, tag="osb")
                            nc.vector.tensor_tensor(o_sb[:], o_ps[qh][0:DK, :],
                                                    rb[:], mybir.AluOpType.mult)
                            nc.sync.dma_start(
                                a2a_in[2 * qs:2 * qs + 2, u, qh].rearrange(
                                    "d p t -> p d t"),
                                o_sb.rearrange("p (d t) -> p d t", d=2))

                nc.gpsimd.collective_compute(
                    "AllToAll", mybir.AluOpType.bypass, replica_groups=RG,
                    ins=[a2a_in[:]], outs=[a2a_out[:]])

                # -------- output projection (+ residual)
                oT_own = qp.tile([128, DSUB, T], bf16, tag="oTown", bufs=1)
                nc.sync.dma_start(
                    oT_own.rearrange("p kt (u t) -> p kt u t", u=B),
                    a2a_out.rearrange("kt u qh dv t -> (qh dv) kt u t"))
                bo_sb = wp.tile([128, DSUB], f32, tag="bo")
                nc.sync.dma_start(bo_sb[:], bo[l])
                for m in range(DSUB):
                    wo_sb = wp.tile([128, DSUB, 128], bf16, tag="wo",
                                    name=f"wo_{l}_{m}")
                    nc.sync.dma_start(wo_sb[:], wo[l, m])
                    wo_ps = pp.tile([128, 512], f32, tag="pp")
                    for kt in range(DSUB):
                        nc.tensor.matmul(wo_ps[:], wo_sb[:, kt, :], oT_own[:, kt, :],
                                         start=(kt == 0), stop=(kt == DSUB - 1))
                    nc.vector.scalar_tensor_tensor(
                        out=xT[:, m, :], in0=wo_ps[:], scalar=bo_sb[:, m:m + 1],
                        in1=xT[:, m, :], op0=mybir.AluOpType.add,
                        op1=mybir.AluOpType.add)

            # ---------------- rmsnorm 2 + FFN
            with tc.tile_pool(name="ffp", bufs=2) as fp, \
                 tc.tile_pool(name="ffw", bufs=3) as fwp, \
                 tc.tile_pool(name="ffps", bufs=4, space="PSUM") as fps, \
                 tc.tile_pool(name="rmps2", bufs=1, space="PSUM") as rps2:
                h2T = rmsnorm(fp, rps2, xT, f"r2_{l}")
                b1_sb = fwp.tile([128, FM], f32, tag="b1")
                b2_sb = fwp.tile([128, FM], f32, tag="b2")
                b3_sb = fwp.tile([128, DSUB], f32, tag="b3")
                nc.sync.dma_start(b1_sb[:], b1[l])
                nc.sync.dma_start(b2_sb[:], b2[l])
                nc.sync.dma_start(b3_sb[:], b3[l])
                fT = fp.tile([128, FM, T], bf16, tag="fT", bufs=1)
                for m in range(FM):
                    w1_sb = fwp.tile([128, DSUB, 128], bf16, tag="w1",
                                     name=f"w1_{l}_{m}")
                    nc.sync.dma_start(w1_sb[:], w1[l, m])
                    a_ps = fps.tile([128, 512], f32, tag="fpp")
                    for o in range(DSUB):
                        nc.tensor.matmul(a_ps[:], w1_sb[:, o, :], h2T[:, o, :],
                                         start=(o == 0), stop=(o == DSUB - 1))
                    aT = fp.tile([128, T], bf16, tag="aT")
                    nc.scalar.activation(aT[:], a_ps[:],
                                         mybir.ActivationFunctionType.Silu,
                                         bias=b1_sb[:, m:m + 1], scale=1.0)
                    w2_sb = fwp.tile([128, DSUB, 128], bf16, tag="w2",
                                     name=f"w2_{l}_{m}")
                    nc.sync.dma_start(w2_sb[:], w2[l, m])
                    b_ps = fps.tile([128, 512], f32, tag="fpp")
                    for o in range(DSUB):
                        nc.tensor.matmul(b_ps[:], w2_sb[:, o, :], h2T[:, o, :],
                                         start=(o == 0), stop=(o == DSUB - 1))
                    nc.vector.scalar_tensor_tensor(
                        out=fT[:, m, :], in0=b_ps[:], scalar=b2_sb[:, m:m + 1],
                        in1=aT[:], op0=mybir.AluOpType.add,
                        op1=mybir.AluOpType.mult)
                for m in range(DSUB):
                    w3_sb = fwp.tile([128, FM, 128], bf16, tag="w3",
                                     name=f"w3_{l}_{m}")
                    nc.sync.dma_start(w3_sb[:], w3[l, m])
                    f_ps = fps.tile([128, 512], f32, tag="fpp")
                    for ko in range(FM):
                        nc.tensor.matmul(f_ps[:], w3_sb[:, ko, :], fT[:, ko, :],
                                         start=(ko == 0), stop=(ko == FM - 1))
                    nc.vector.scalar_tensor_tensor(
                        out=xT[:, m, :], in0=f_ps[:], scalar=b3_sb[:, m:m + 1],
                        in1=xT[:, m, :], op0=mybir.AluOpType.add,
                        op1=mybir.AluOpType.add)

        # ---------------- final norm + AllGather + vocab-sharded head
        embH_sb = embp.tile([128, DSUB, VSHP], bf16)
        nc.sync.dma_start(embH_sb[:], embH)
        with tc.tile_pool(name="fin", bufs=1) as fin, \
             tc.tile_pool(name="fips", bufs=1, space="PSUM") as fips:
            hfT = rmsnorm(fin, fips, xT, "rf")
            agf_in = dram.tile([D, T], bf16, tag="ag_in")
            nc.sync.dma_start(agf_in.rearrange("(o p) t -> p o t", p=128), hfT[:])
            hf_all = dram.tile([N_CORES * D, T], bf16, tag="hT_all",
                               addr_space="Shared")
            nc.gpsimd.collective_compute(
                "AllGather", mybir.AluOpType.bypass, replica_groups=RG,
                ins=[agf_in[:]], outs=[hf_all[:]])
            hf_view = hf_all.rearrange("(r d) t -> r d t", r=N_CORES)

        with tc.tile_pool(name="hd", bufs=2) as hd, \
             tc.tile_pool(name="hdps", bufs=8, space="PSUM") as hdps:
            for r in range(N_CORES):
                hf_r = hd.tile([128, DSUB, T], bf16, tag="hfr", name=f"hfr_{r}")
                nc.sync.dma_start(
                    hf_r[:], hf_view[r].rearrange("(o p) t -> p o t", p=128))
                for mt in range(4):
                    lo_ps = [hdps.tile([128, 512], f32, tag="lo",
                                       name=f"lo_{r}_{mt}_{nq}")
                             for nq in range(8)]
                    for o in range(DSUB):
                        for nq in range(8):
                            nc.tensor.matmul(
                                lo_ps[nq][:], hf_r[:, o, 128 * mt:128 * (mt + 1)],
                                embH_sb[:, o, 512 * nq:512 * (nq + 1)],
                                start=(o == 0), stop=(o == DSUB - 1))
                    for nq in range(8):
                        lo_sb = hd.tile([128, 512], f32, tag="losb",
                                        name=f"losb_{r}_{mt}_{nq}", bufs=6)
                        if nq % 2 == 0:
                            nc.vector.tensor_copy(lo_sb[:], lo_ps[nq][:])
                        else:
                            nc.scalar.copy(lo_sb[:], lo_ps[nq][:])
                        nc.sync.dma_start(
                            logits[512 * r + 128 * mt:512 * r + 128 * (mt + 1),
                                   512 * nq:512 * (nq + 1)],
                            lo_sb[:])

        dram.release(); embp.release(); xp.release(); const.release()

    nc.compile()
    return nc


# ---------------------------------------------------------------- host-side prep

def _deinterleave(w):
    """Permute last-dim head features to [evens | odds] per 64-wide head."""
    out = np.empty_like(w)
    nh = w.shape[-1] // DK
    for h in range(nh):
        blk = w[..., h * DK:(h + 1) * DK]
        out[..., h * DK:h * DK + 32] = blk[..., 0::2]
        out[..., h * DK + 32:(h + 1) * DK] = blk[..., 1::2]
    return out


def _prep_inputs(inputs):
    """Build per-core in_maps (host-side numpy preprocessing)."""
    tokens = np.asarray(inputs["tokens"])
    embed = np.asarray(inputs["embed"], np.float32)
    g1 = np.asarray(inputs["g1"], np.float32)
    g2 = np.asarray(inputs["g2"], np.float32)
    gpost = np.asarray(inputs["gpost"], np.float32)
    fcos = np.asarray(inputs["freqs_cos"], np.float32)   # [S, 32]
    fsin = np.asarray(inputs["freqs_sin"], np.float32)

    wq_f = np.asarray(inputs["wq"], np.float32) * g1[:, :, None]
    wk_f = np.asarray(inputs["wk"], np.float32) * g1[:, :, None]
    wv_f = np.asarray(inputs["wv"], np.float32) * g1[:, :, None]
    wq_f = _deinterleave(wq_f) * (1.0 / np.sqrt(DK, dtype=np.float32))
    wk_f = _deinterleave(wk_f)
    bq_f = _deinterleave(np.asarray(inputs["bq"], np.float32)) / np.sqrt(DK)
    bk_f = _deinterleave(np.asarray(inputs["bk"], np.float32))
    bv_f = np.asarray(inputs["bv"], np.float32)
    wo_f = np.asarray(inputs["wo"], np.float32)
    bo_f = np.asarray(inputs["bo"], np.float32)
    w1_f = np.asarray(inputs["w1"], np.float32) * g2[:, :, None]
    w2_f = np.asarray(inputs["w2"], np.float32) * g2[:, :, None]
    w3_f = np.asarray(inputs["w3"], np.float32)
    b1_f = np.asarray(inputs["b1"], np.float32)
    b2_f = np.asarray(inputs["b2"], np.float32)
    b3_f = np.asarray(inputs["b3"], np.float32)

    # shared tensors
    w1p = np.zeros((L, D, FDP), np.float32); w1p[:, :, :FD] = w1_f
    w2p = np.zeros((L, D, FDP), np.float32); w2p[:, :, :FD] = w2_f
    w3p = np.zeros((L, FDP, D), np.float32); w3p[:, :FD, :] = w3_f
    b1p = np.zeros((L, FDP), np.float32); b1p[:, :FD] = b1_f
    b2p = np.zeros((L, FDP), np.float32); b2p[:, :FD] = b2_f

    # pre-tiled shared weights
    wo_t = np.ascontiguousarray(
        wo_f.reshape(L, DSUB, 128, DSUB, 128)      # l, kt, p, m, c
            .transpose(0, 3, 2, 1, 4))             # l, m, p, kt, c
    w1_t = np.ascontiguousarray(
        w1p.reshape(L, DSUB, 128, FM, 128)         # l, o, p, m, c
           .transpose(0, 3, 2, 1, 4))              # l, m, p, o, c
    w2_t = np.ascontiguousarray(
        w2p.reshape(L, DSUB, 128, FM, 128).transpose(0, 3, 2, 1, 4))
    w3_t = np.ascontiguousarray(
        w3p.reshape(L, FM, 128, DSUB, 128)         # l, ko, p, m, c
           .transpose(0, 3, 2, 1, 4))              # l, m, p, ko, c
    bo_t = np.ascontiguousarray(bo_f.reshape(L, DSUB, 128).transpose(0, 2, 1))
    b1_t = np.ascontiguousarray(b1p.reshape(L, FM, 128).transpose(0, 2, 1))
    b2_t = np.ascontiguousarray(b2p.reshape(L, FM, 128).transpose(0, 2, 1))
    b3_t = np.ascontiguousarray(b3_f.reshape(L, DSUB, 128).transpose(0, 2, 1))

    # rope tables
    c1_t = np.empty((DK, TB), np.float32)
    c2_t = np.empty((DK, TB), np.float32)
    c1_t[0:32] = fcos.T; c1_t[32:64] = fcos.T
    c2_t[0:32] = -fsin.T; c2_t[32:64] = fsin.T
    psw = np.zeros((DK, DK), np.float32)
    for i in range(32):
        psw[i, i + 32] = 1.0
        psw[i + 32, i] = 1.0

    embT = embed.T * gpost[:, None]                # [D, V]

    x_emb = embed[tokens]                          # [B, S, D]

    in_maps = []
    for c in range(N_CORES):
        # x0T: feature-major [128, DSUB, T] for this core's tokens
        xc = x_emb[:, TC * c:TC * (c + 1), :]      # [B, TC, D]
        x0T = np.ascontiguousarray(
            xc.reshape(B * TC, D).T.reshape(DSUB, 128, T).transpose(1, 0, 2))
        wq_c = np.ascontiguousarray(
            wq_f[:, :, 128 * c:128 * (c + 1)]
            .reshape(L, DSUB, 128, 128).transpose(0, 2, 1, 3))   # l p o m
        wk_c = np.ascontiguousarray(
            wk_f[:, :, DK * c:DK * (c + 1)]
            .reshape(L, DSUB, 128, DK).transpose(0, 2, 1, 3))
        wv_c = np.ascontiguousarray(
            wv_f[:, :, DK * c:DK * (c + 1)]
            .reshape(L, DSUB, 128, DK).transpose(0, 2, 1, 3))
        embH_c = np.zeros((D, VSHP), np.float32)
        embH_c[:, :VSH] = embT[:, VSH * c:VSH * (c + 1)]
        embH_t = np.ascontiguousarray(
            embH_c.reshape(DSUB, 128, VSHP).transpose(1, 0, 2))
        bvB_c = np.broadcast_to(bv_f[:, None, DK * c:DK * (c + 1)],
                                (L, 128, DK)).copy()
        in_maps.append({
            "x0T": x0T.astype(np.float32),
            "wq": wq_c.astype(BF), "wk": wk_c.astype(BF), "wv": wv_c.astype(BF),
            "wo": wo_t.astype(BF), "w1": w1_t.astype(BF), "w2": w2_t.astype(BF),
            "w3": w3_t.astype(BF), "embH": embH_t.astype(BF),
            "c1": c1_t, "c2": c2_t, "pswap": psw.astype(BF),
            "ident": np.eye(DK, dtype=np.float32).astype(BF),
            "bq": np.ascontiguousarray(bq_f[:, 128 * c:128 * (c + 1)]).astype(np.float32),
            "bk": np.ascontiguousarray(bk_f[:, DK * c:DK * (c + 1)]).astype(np.float32),
            "bvB": bvB_c.astype(np.float32), "bo": bo_t, "b1": b1_t,
            "b2": b2_t, "b3": b3_t,
        })
    return in_maps


# ---------------------------------------------------------------- runner

def _make_runner(nc):
    """Cached sharded executable (mirrors bass2jax.run_bass_via_pjrt)."""
    import jax
    from jax.sharding import Mesh, PartitionSpec
    from jax.experimental.shard_map import shard_map

    bass2jax.install_neuronx_cc_hook()
    in_names, out_names, out_avals, zero_shapes = [], [], [], []
    pid_name = nc.partition_id_tensor.name if nc.partition_id_tensor else None
    for alloc in nc.m.functions[0].allocations:
        if not isinstance(alloc, mybir.MemoryLocationSet):
            continue
        name = alloc.memorylocations[0].name
        if alloc.kind == "ExternalInput":
            if name != pid_name:
                in_names.append(name)
        elif alloc.kind == "ExternalOutput":
            out_names.append(name)
            shape = tuple(alloc.tensor_shape)
            dtype = mybir.dt.np(alloc.dtype)
            out_avals.append(jax.core.ShapedArray(shape, dtype))
            zero_shapes.append((shape, dtype))
    n_params = len(in_names)
    all_names = list(in_names) + list(out_names)
    if pid_name is not None:
        all_names.append(pid_name)

    def _body(*args):
        operands = list(args)
        if pid_name is not None:
            operands.append(bass2jax.partition_id_tensor())
        outs = bass2jax._bass_exec_p.bind(
            *operands, out_avals=tuple(out_avals), in_names=tuple(all_names),
            out_names=tuple(out_names), lowering_input_output_aliases=(),
            sim_require_finite=True, sim_require_nnan=True, nc=nc)
        return tuple(outs)

    devices = jax.devices()[:N_CORES]
    mesh = Mesh(np.asarray(devices), ("core",))
    n_outs = len(out_names)
    sharded = jax.jit(
        shard_map(_body, mesh=mesh,
                  in_specs=(PartitionSpec("core"),) * (n_params + n_outs),
                  out_specs=(PartitionSpec("core"),) * n_outs,
                  check_rep=False),
        donate_argnums=tuple(range(n_params, n_params + n_outs)),
        keep_unused=True)
    return sharded, in_names, out_names, zero_shapes


def _get_exec():
    if "exec" not in _CACHE:
        nc = build_program()
        _CACHE["exec"] = _make_runner(nc)
    return _CACHE["exec"]


def run_device(in_maps):
    """Run the SPMD program; returns list of per-core output dicts."""
    import jax
    sharded, in_names, out_names, zero_shapes = _get_exec()
    concat_in = [np.concatenate([m[n] for m in in_maps], axis=0)
                 for n in in_names]
    zeros = [np.zeros((N_CORES * s[0], *s[1:]), d) for s, d in zero_shapes]
    outs = sharded(*concat_in, *zeros)
    outs = [np.asarray(o) for o in outs]
    res = []
    for c in range(N_CORES):
        res.append({n: outs[i].reshape(N_CORES, *zero_shapes[i][0])[c]
                    for i, n in enumerate(out_names)})
    return res


# ---------------------------------------------------------------- entry point

def kernel(**inputs):
    in_maps = _prep_inputs(inputs)
    res = run_device(in_maps)
    # assemble [B, S, V] from per-core vocab shards of all tokens
    logits = np.empty((B, S, V), np.float32)
    for c in range(N_CORES):
        lc = res[c]["logits"]                       # [4096, VSHP]
        lc = lc[:, :VSH].reshape(N_CORES, B, TC, VSH)
        for r in range(N_CORES):
            logits[:, TC * r:TC * (r + 1), VSH * c:VSH * (c + 1)] = lc[r]
    return logits


# revision 10
# speedup vs baseline: 329.4646x; 329.4646x over previous
"""Trainium2 Bass kernel for nn_CausalLM_36910948942461.

4-layer causal LM: D=1024, H=16 (GQA KVH=8, dk=64), SwiGLU FFN 2752, S=2048, B=2,
V=32000, tied embedding head, full logits out [2, 2048, 32000] fp32.

Sharding over 8 NeuronCores:
  - Attention: by (batch, head-pair). Core c owns q-heads {2c, 2c+1} + kv-head c for
    BOTH batches over the full 2048-token sequence (perfectly uniform SPMD).
  - Everything else (residual stream, norms, QKV/out projections, FFN): by token.
    Core c owns positions [256c, 256c+256) of each batch (512 tokens).
  - LM head: vocab-sharded (core c computes logit columns [4000c, 4000c+4000) for
    all 4096 tokens); host concatenates.
Per layer: AllGather of normalized activations (1 MB/rank) before QKV, AllToAll of
attention outputs (1 MB/rank) before the output projection.
Compute: bf16 matmuls with fp32 PSUM accumulation; fp32 residual stream.
RoPE: weight columns host-permuted to deinterleaved (r|i) layout; rotation on device
via a partition-swap permutation matmul + 3 elementwise ops with host cos/sin tables.
Softmax: scores computed transposed [key, query]; exp on ScalarE (no max subtraction:
logit range is tiny for this model); denominators via a ones-augmented V column.
"""

import numpy as np
import ml_dtypes

import concourse.bass as bass
import concourse.mybir as mybir
import concourse.tile as tile
from concourse import bacc, bass2jax

f32 = mybir.dt.float32
bf16 = mybir.dt.bfloat16
BF = ml_dtypes.bfloat16

# Model dims (hardcoded per problem spec)
N_CORES = 8
V, L, D, H, KVH, FD, S, B = 32000, 4, 1024, 16, 8, 2752, 2048, 2
DK = 64                      # head dim
DSUB = D // 128              # 8 partition tiles of the model dim
TB = S                       # tokens per batch (full sequence)
TC = S // N_CORES            # 256 tokens per (core, batch)
T = B * TC                   # 512 tokens owned per core
FDP = 2816                   # FD padded to 22*128
FM = FDP // 128              # 22
VSH = V // N_CORES           # 4000 vocab columns per core
VSHP = 4096                  # padded
EPS = 1e-6

_CACHE = {}


# ---------------------------------------------------------------- device program

def build_program():
    nc = bacc.Bacc("TRN2", target_bir_lowering=False, debug=False,
                   enable_asserts=True, num_devices=N_CORES)

    def din(name, shape, dt=bf16):
        return nc.dram_tensor(name, shape, dt, kind="ExternalInput").ap()

    # Inputs (per core). Weight layouts are host-pre-tiled for contiguous DMA.
    x0T = din("x0T", [128, DSUB, T], f32)          # embedded tokens, feature-major
    wq = din("wq", [L, 128, DSUB, 128])            # [l, p, o, m] q-head pair cols (perm, g1, /8)
    wk = din("wk", [L, 128, DSUB, DK])             # kv-head cols (perm, g1)
    wv = din("wv", [L, 128, DSUB, DK])             # (g1)
    wo = din("wo", [L, DSUB, 128, DSUB, 128])      # [l, m, p, kt, c]
    w1 = din("w1", [L, FM, 128, DSUB, 128])        # [l, m, p, o, c] (g2, padded)
    w2 = din("w2", [L, FM, 128, DSUB, 128])
    w3 = din("w3", [L, DSUB, 128, FM, 128])        # [l, m, p, ko, c] (padded)
    embH = din("embH", [128, DSUB, VSHP])          # head matrix slice (gpost folded)
    c1 = din("c1", [DK, TB], f32)                  # rope cos table (stacked x2)
    c2 = din("c2", [DK, TB], f32)                  # rope sin table (-sin | +sin)
    pswap = din("pswap", [DK, DK])                 # 32-half swap permutation
    ident = din("ident", [DK, DK])                 # 64x64 identity (for PE transpose)
    bq = din("bq", [L, 128], f32)
    bk = din("bk", [L, DK], f32)
    bvB = din("bvB", [L, 128, DK], f32)            # bv replicated over partitions
    bo = din("bo", [L, 128, DSUB], f32)
    b1 = din("b1", [L, 128, FM], f32)
    b2 = din("b2", [L, 128, FM], f32)
    b3 = din("b3", [L, 128, DSUB], f32)

    logits = nc.dram_tensor("logits", [N_CORES * T, VSHP], f32,
                            kind="ExternalOutput").ap()

    RG = [list(range(N_CORES))]

    with tile.TileContext(nc) as tc:
        # ---- persistent pools
        const = tc.alloc_tile_pool(name="const", bufs=1)
        xp = tc.alloc_tile_pool(name="xp", bufs=1)
        embp = tc.alloc_tile_pool(name="embp", bufs=1)
        dram = tc.alloc_tile_pool(name="dram", bufs=2, space="DRAM")

        c1_sb = const.tile([DK, TB], f32)
        c2_sb = const.tile([DK, TB], f32)
        pswap_sb = const.tile([DK, DK], bf16)
        ident64 = const.tile([DK, DK], bf16)
        ones_pcol = const.tile([128, 1], bf16)     # partition-sum lhsT
        ones_row = const.tile([1, 128], f32)       # K=1 broadcast lhsT
        nc.sync.dma_start(c1_sb[:], c1)
        nc.sync.dma_start(c2_sb[:], c2)
        nc.sync.dma_start(pswap_sb[:], pswap)
        nc.sync.dma_start(ident64[:], ident)
        eps_sb = const.tile([1, 1], f32)
        nc.any.memset(ones_pcol[:], 1.0)
        nc.any.memset(ones_row[:], 1.0)
        nc.any.memset(eps_sb[:], EPS)

        xT = xp.tile([128, DSUB, T], f32)
        nc.sync.dma_start(xT[:], x0T)

        def rmsnorm(pool, psum, src, name):
            """src [128, DSUB, T] f32 -> normalized bf16 (gains folded into weights)."""
            xsq = pool.tile([128, DSUB, T], bf16, tag="xsq", name=f"xsq_{name}", bufs=1)
            nc.scalar.activation(xsq[:], src[:], mybir.ActivationFunctionType.Square)
            ssq = psum.tile([1, T], f32, tag="ssq", name=f"ssq_{name}")
            for o in range(DSUB):
                nc.tensor.matmul(ssq[:], ones_pcol[:], xsq[:, o, :],
                                 start=(o == 0), stop=(o == DSUB - 1))
            srt = pool.tile([1, T], f32, tag="srt", name=f"srt_{name}")
            nc.scalar.activation(srt[:], ssq[:], mybir.ActivationFunctionType.Sqrt,
                                 scale=1.0 / D, bias=eps_sb[:])
            rs = pool.tile([1, T], f32, tag="rs", name=f"rs_{name}")
            nc.vector.reciprocal(rs[:], srt[:])
            rsB = psum.tile([128, T], f32, tag="rsB", name=f"rsB_{name}")
            nc.tensor.matmul(rsB[:], ones_row[:], rs[:], start=True, stop=True)
            out = pool.tile([128, DSUB, T], bf16, tag="hT", name=f"hT_{name}", bufs=1)
            nc.vector.tensor_tensor(
                out[:], src[:], rsB[:, None, :].to_broadcast((128, DSUB, T)),
                mybir.AluOpType.mult)
            return out

        hT_all_view = None
        for l in range(L):
            # ---------------- rmsnorm 1 + AllGather of normalized activations
            with tc.tile_pool(name="rmsp", bufs=1) as rp, \
                 tc.tile_pool(name="rmps", bufs=1, space="PSUM") as rps:
                hT = rmsnorm(rp, rps, xT, f"r1_{l}")
                ag_in = dram.tile([D, T], bf16, tag="ag_in")
                nc.sync.dma_start(ag_in.rearrange("(o p) t -> p o t", p=128), hT[:])
                hT_all = dram.tile([N_CORES * D, T], bf16, tag="hT_all",
                                   addr_space="Shared")
                nc.gpsimd.collective_compute(
                    "AllGather", mybir.AluOpType.bypass, replica_groups=RG,
                    ins=[ag_in[:]], outs=[hT_all[:]])
                hT_all_view = hT_all.rearrange("(r d) t -> r d t", r=N_CORES)

            # layer-scope pools for qkv + attention
            with tc.tile_pool(name="wqkv", bufs=2) as wp, \
                 tc.tile_pool(name="htb", bufs=2) as hp, \
                 tc.tile_pool(name="qkv", bufs=2) as qp, \
                 tc.tile_pool(name="attn", bufs=3) as ap, \
                 tc.tile_pool(name="pp", bufs=2, space="PSUM") as pp, \
                 tc.tile_pool(name="scp", bufs=1, space="PSUM") as scp, \
                 tc.tile_pool(name="op0", bufs=2, space="PSUM") as opp0, \
                 tc.tile_pool(name="op1", bufs=2, space="PSUM") as opp1, \
                 tc.tile_pool(name="a2ap", bufs=1, space="DRAM") as a2ap:

                wq_sb = wp.tile([128, DSUB, 128], bf16, tag="wq")
                wk_sb = wp.tile([128, DSUB, DK], bf16, tag="wk")
                wv_sb = wp.tile([128, DSUB, DK], bf16, tag="wv")
                nc.sync.dma_start(wq_sb[:], wq[l])
                nc.sync.dma_start(wk_sb[:], wk[l])
                nc.sync.dma_start(wv_sb[:], wv[l])
                bq_sb = wp.tile([128, 1], f32, tag="bq")
                bk_sb = wp.tile([DK, 1], f32, tag="bk")
                bvB_sb = wp.tile([128, DK], f32, tag="bvB")
                nc.sync.dma_start(bq_sb[:], bq[l][:, None])
                nc.sync.dma_start(bk_sb[:], bk[l][:, None])
                nc.sync.dma_start(bvB_sb[:], bvB[l])

                a2a_in = a2ap.tile([N_CORES, B, 2, DK, TC], bf16, tag="a2a_in")
                a2a_out = a2ap.tile([N_CORES, B, 2, DK, TC], bf16, tag="a2a_out")

                for u in range(B):
                    # -------- q/k/v projections over this batch's 2048 tokens
                    qT = qp.tile([DK, 2, TB], bf16, tag="qT", bufs=1)
                    kT = qp.tile([DK, TB], bf16, tag="kT", bufs=1)
                    v_aug = qp.tile([128, TB // 128, DK + 1], bf16, tag="vaug")
                    nc.any.memset(v_aug[:, :, DK:DK + 1], 1.0)
                    for n in range(4):  # 512-token chunks of this batch
                        htb = hp.tile([128, DSUB, 512], bf16, tag="htb")
                        for rr in range(2):
                            nc.sync.dma_start(
                                htb[:, :, 256 * rr:256 * (rr + 1)],
                                hT_all_view[2 * n + rr, :, TC * u:TC * (u + 1)]
                                .rearrange("(o p) t -> p o t", p=128))
                        cs = slice(512 * n, 512 * (n + 1))
                        q_ps = pp.tile([128, 512], f32, tag="pp")
                        for o in range(DSUB):
                            nc.tensor.matmul(q_ps[:], wq_sb[:, o, :], htb[:, o, :],
                                             start=(o == 0), stop=(o == DSUB - 1))
                        nc.vector.tensor_scalar_add(qT[:, 0, cs], q_ps[0:DK],
                                                    bq_sb[0:DK])
                        nc.vector.tensor_scalar_add(qT[:, 1, cs], q_ps[DK:128],
                                                    bq_sb[DK:128])
                        k_ps = pp.tile([DK, 512], f32, tag="pp")
                        for o in range(DSUB):
                            nc.tensor.matmul(k_ps[:], wk_sb[:, o, :], htb[:, o, :],
                                             start=(o == 0), stop=(o == DSUB - 1))
                        nc.vector.tensor_scalar_add(kT[:, cs], k_ps[:], bk_sb[:])
                        vT_ps = pp.tile([DK, 512], f32, tag="pp")
                        for o in range(DSUB):
                            nc.tensor.matmul(vT_ps[:], wv_sb[:, o, :], htb[:, o, :],
                                             start=(o == 0), stop=(o == DSUB - 1))
                        vT_sb = hp.tile([DK, 512], bf16, tag="vT")
                        nc.scalar.copy(vT_sb[:], vT_ps[:])
                        for sblk in range(4):
                            vtr = pp.tile([128, DK], bf16, tag="pp")
                            nc.tensor.transpose(vtr[:], vT_sb[:, 128 * sblk:128 * (sblk + 1)],
                                                ident64[:])
                            nc.vector.tensor_tensor(
                                v_aug[:, 4 * n + sblk, 0:DK], vtr[:], bvB_sb[:],
                                mybir.AluOpType.add)

                    # -------- RoPE (q and k)
                    qrT = qp.tile([DK, 2, TB], bf16, tag="qrT")
                    q_flat = qT.rearrange("p a t -> p (a t)")
                    qr_flat = qrT.rearrange("p a t -> p (a t)")
                    for c8 in range(8):
                        fs = slice(512 * c8, 512 * (c8 + 1))
                        ts_ = slice(512 * (c8 % 4), 512 * (c8 % 4 + 1))
                        sw_ps = pp.tile([DK, 512], f32, tag="pp")
                        nc.tensor.matmul(sw_ps[:], pswap_sb[:], q_flat[:, fs],
                                         start=True, stop=True)
                        tmp = hp.tile([DK, 512], bf16, tag="ropetmp")
                        nc.vector.tensor_tensor(tmp[:], sw_ps[:], c2_sb[:, ts_],
                                                mybir.AluOpType.mult)
                        nc.vector.tensor_tensor(qr_flat[:, fs], q_flat[:, fs],
                                                c1_sb[:, ts_], mybir.AluOpType.mult)
                        nc.vector.tensor_tensor(qr_flat[:, fs], qr_flat[:, fs],
                                                tmp[:], mybir.AluOpType.add)
                    krT = qp.tile([DK, TB], bf16, tag="krT")
                    for c4 in range(4):
                        fs = slice(512 * c4, 512 * (c4 + 1))
                        sw_ps = pp.tile([DK, 512], f32, tag="pp")
                        nc.tensor.matmul(sw_ps[:], pswap_sb[:], kT[:, fs],
                                         start=True, stop=True)
                        tmp = hp.tile([DK, 512], bf16, tag="ropetmp")
                        nc.vector.tensor_tensor(tmp[:], sw_ps[:], c2_sb[:, fs],
                                                mybir.AluOpType.mult)
                        nc.vector.tensor_tensor(krT[:, fs], kT[:, fs],
                                                c1_sb[:, fs], mybir.AluOpType.mult)
                        nc.vector.tensor_tensor(krT[:, fs], krT[:, fs],
                                                tmp[:], mybir.AluOpType.add)

                    # -------- attention: scoresT -> exp -> (mask) -> oT accumulate
                    opps = [opp0, opp1]
                    for qs in range(4):
                        o_ps = [opps[qh].tile([128, 512], f32, tag=f"ops{qh}",
                                              name=f"ops{qh}_{l}_{u}_{qs}")
                                for qh in range(2)]
                        nj = 4 * qs + 4
                        for j in range(nj):
                            doff = max(0, 128 * (j - 4 * qs))
                            tq = 512 - doff
                            sc_ps = scp.tile([128, 2, 512], f32, tag="sc",
                                             name=f"sc_{l}_{u}_{qs}_{j}")
                            for qh in range(2):
                                nc.tensor.matmul(
                                    sc_ps[:, qh, 0:tq],
                                    krT[:, 128 * j:128 * (j + 1)],
                                    qrT[:, qh, 512 * qs + doff:512 * (qs + 1)],
                                    start=True, stop=True)
                            ex = ap.tile([128, 2, 512], bf16, tag="exp",
                                         name=f"ex_{l}_{u}_{qs}_{j}")
                            nc.scalar.activation(ex[:, :, 0:tq], sc_ps[:, :, 0:tq],
                                                 mybir.ActivationFunctionType.Exp)
                            if j >= 4 * qs:
                                for qh in range(2):
                                    nc.gpsimd.affine_select(
                                        ex[:, qh, 0:128], ex[:, qh, 0:128],
                                        pattern=[[1, 128]],
                                        compare_op=mybir.AluOpType.is_ge,
                                        fill=0.0, base=0, channel_multiplier=-1)
                            for qh in range(2):
                                nc.tensor.matmul(
                                    o_ps[qh][0:DK + 1, doff:512],
                                    v_aug[:, j, :], ex[:, qh, 0:tq],
                                    start=(j == 0), stop=(j == nj - 1))
                        for qh in range(2):
                            recip = ap.tile([1, 512], f32, tag="recip")
                            nc.vector.reciprocal(recip[:], o_ps[qh][DK:DK + 1, :])
                            rb = ap.tile([DK, 512], f32, tag="rb")
                            nc.gpsimd.partition_broadcast(rb[:], recip[:])
                            o_sb = ap.tile([DK, 512], bf16, tag="osb")
                            nc.vector.tensor_tensor(o_sb[:], o_ps[qh][0:DK, :],
                                                    rb[:], mybir.AluOpType.mult)
                            nc.sync.dma_start(
                                a2a_in[2 * qs:2 * qs + 2, u, qh].rearrange(
                                    "d p t -> p d t"),
                                o_sb.rearrange("p (d t) -> p d t", d=2))

                nc.gpsimd.collective_compute(
                    "AllToAll", mybir.AluOpType.bypass, replica_groups=RG,
                    ins=[a2a_in[:]], outs=[a2a_out[:]])

                # -------- output projection (+ residual)
                oT_own = qp.tile([128, DSUB, T], bf16, tag="oTown", bufs=1)
                nc.sync.dma_start(
                    oT_own.rearrange("p kt (u t) -> p kt u t", u=B),
                    a2a_out.rearrange("kt u qh dv t -> (qh dv) kt u t"))
                bo_sb = wp.tile([128, DSUB], f32, tag="bo")
                nc.sync.dma_start(bo_sb[:], bo[l])
                for m in range(DSUB):
                    wo_sb = wp.tile([128, DSUB, 128], bf16, tag="wo",
                                    name=f"wo_{l}_{m}")
                    nc.sync.dma_start(wo_sb[:], wo[l, m])
                    wo_ps = pp.tile([128, 512], f32, tag="pp")
                    for kt in range(DSUB):
                        nc.tensor.matmul(wo_ps[:], wo_sb[:, kt, :], oT_own[:, kt, :],
                                         start=(kt == 0), stop=(kt == DSUB - 1))
                    nc.vector.scalar_tensor_tensor(
                        out=xT[:, m, :], in0=wo_ps[:], scalar=bo_sb[:, m:m + 1],
                        in1=xT[:, m, :], op0=mybir.AluOpType.add,
                        op1=mybir.AluOpType.add)

            # ---------------- rmsnorm 2 + FFN
            with tc.tile_pool(name="ffp", bufs=2) as fp, \
                 tc.tile_pool(name="ffw", bufs=3) as fwp, \
                 tc.tile_pool(name="ffps", bufs=4, space="PSUM") as fps, \
                 tc.tile_pool(name="rmps2", bufs=1, space="PSUM") as rps2:
                h2T = rmsnorm(fp, rps2, xT, f"r2_{l}")
                b1_sb = fwp.tile([128, FM], f32, tag="b1")
                b2_sb = fwp.tile([128, FM], f32, tag="b2")
                b3_sb = fwp.tile([128, DSUB], f32, tag="b3")
                nc.sync.dma_start(b1_sb[:], b1[l])
                nc.sync.dma_start(b2_sb[:], b2[l])
                nc.sync.dma_start(b3_sb[:], b3[l])
                fT = fp.tile([128, FM, T], bf16, tag="fT", bufs=1)
                for m in range(FM):
                    w1_sb = fwp.tile([128, DSUB, 128], bf16, tag="w1",
                                     name=f"w1_{l}_{m}")
                    nc.sync.dma_start(w1_sb[:], w1[l, m])
                    a_ps = fps.tile([128, 512], f32, tag="fpp")
                    for o in range(DSUB):
                        nc.tensor.matmul(a_ps[:], w1_sb[:, o, :], h2T[:, o, :],
                                         start=(o == 0), stop=(o == DSUB - 1))
                    aT = fp.tile([128, T], bf16, tag="aT")
                    nc.scalar.activation(aT[:], a_ps[:],
                                         mybir.ActivationFunctionType.Silu,
                                         bias=b1_sb[:, m:m + 1], scale=1.0)
                    w2_sb = fwp.tile([128, DSUB, 128], bf16, tag="w2",
                                     name=f"w2_{l}_{m}")
                    nc.sync.dma_start(w2_sb[:], w2[l, m])
                    b_ps = fps.tile([128, 512], f32, tag="fpp")
                    for o in range(DSUB):
                        nc.tensor.matmul(b_ps[:], w2_sb[:, o, :], h2T[:, o, :],
                                         start=(o == 0), stop=(o == DSUB - 1))
                    nc.vector.scalar_tensor_tensor(
                        out=fT[:, m, :], in0=b_ps[:], scalar=b2_sb[:, m:m + 1],
                        in1=aT[:], op0=mybir.AluOpType.add,
                        op1=mybir.AluOpType.mult)
                for m in range(DSUB):
                    w3_sb = fwp.tile([128, FM, 128], bf16, tag="w3",
                                     name=f"w3_{l}_{m}")
                    nc.sync.dma_start(w3_sb[:], w3[l, m])
                    f_ps = fps.tile([128, 512], f32, tag="fpp")
                    for ko in range(FM):
                        nc.tensor.matmul(f_ps[:], w3_sb[:, ko, :], fT[:, ko, :],
                                         start=(ko == 0), stop=(ko == FM - 1))
                    nc.vector.scalar_tensor_tensor(
                        out=xT[:, m, :], in0=f_ps[:], scalar=b3_sb[:, m:m + 1],
                        in1=xT[:, m, :], op0=mybir.AluOpType.add,
                        op1=mybir.AluOpType.add)

        # ---------------- final norm + AllGather + vocab-sharded head
        embH_sb = embp.tile([128, DSUB, VSHP], bf16)
        nc.sync.dma_start(embH_sb[:], embH)
        with tc.tile_pool(name="fin", bufs=1) as fin, \
             tc.tile_pool(name="fips", bufs=1, space="PSUM") as fips:
            hfT = rmsnorm(fin, fips, xT, "rf")
            agf_in = dram.tile([D, T], bf16, tag="ag_in")
            nc.sync.dma_start(agf_in.rearrange("(o p) t -> p o t", p=128), hfT[:])
            hf_all = dram.tile([N_CORES * D, T], bf16, tag="hT_all",
                               addr_space="Shared")
            nc.gpsimd.collective_compute(
                "AllGather", mybir.AluOpType.bypass, replica_groups=RG,
                ins=[agf_in[:]], outs=[hf_all[:]])
            hf_view = hf_all.rearrange("(r d) t -> r d t", r=N_CORES)

        with tc.tile_pool(name="hd", bufs=2) as hd, \
             tc.tile_pool(name="hdps", bufs=8, space="PSUM") as hdps:
            for r in range(N_CORES):
                hf_r = hd.tile([128, DSUB, T], bf16, tag="hfr", name=f"hfr_{r}")
                nc.sync.dma_start(
                    hf_r[:], hf_view[r].rearrange("(o p) t -> p o t", p=128))
                for mt in range(4):
                    lo_ps = [hdps.tile([128, 512], f32, tag="lo",
                                       name=f"lo_{r}_{mt}_{nq}")
                             for nq in range(8)]
                    for o in range(DSUB):
                        for nq in range(8):
                            nc.tensor.matmul(
                                lo_ps[nq][:], hf_r[:, o, 128 * mt:128 * (mt + 1)],
                                embH_sb[:, o, 512 * nq:512 * (nq + 1)],
                                start=(o == 0), stop=(o == DSUB - 1))
                    for nq in range(8):
                        lo_sb = hd.tile([128, 512], f32, tag="losb",
                                        name=f"losb_{r}_{mt}_{nq}", bufs=6)
                        if nq % 2 == 0:
                            nc.vector.tensor_copy(lo_sb[:], lo_ps[nq][:])
                        else:
                            nc.scalar.copy(lo_sb[:], lo_ps[nq][:])
                        nc.sync.dma_start(
                            logits[512 * r + 128 * mt:512 * r + 128 * (mt + 1),
                                   512 * nq:512 * (nq + 1)],
                            lo_sb[:])

        dram.release(); embp.release(); xp.release(); const.release()

    nc.compile()
    return nc


# ---------------------------------------------------------------- host-side prep

def _deinterleave(w):
    """Permute last-dim head features to [evens | odds] per 64-wide head."""
    out = np.empty_like(w)
    nh = w.shape[-1] // DK
    for h in range(nh):
        blk = w[..., h * DK:(h + 1) * DK]
        out[..., h * DK:h * DK + 32] = blk[..., 0::2]
        out[..., h * DK + 32:(h + 1) * DK] = blk[..., 1::2]
    return out


def _prep_inputs(inputs):
    """Build per-core in_maps (host-side numpy preprocessing)."""
    tokens = np.asarray(inputs["tokens"])
    embed = np.asarray(inputs["embed"], np.float32)
    g1 = np.asarray(inputs["g1"], np.float32)
    g2 = np.asarray(inputs["g2"], np.float32)
    gpost = np.asarray(inputs["gpost"], np.float32)
    fcos = np.asarray(inputs["freqs_cos"], np.float32)   # [S, 32]
    fsin = np.asarray(inputs["freqs_sin"], np.float32)

    wq_f = np.asarray(inputs["wq"], np.float32) * g1[:, :, None]
    wk_f = np.asarray(inputs["wk"], np.float32) * g1[:, :, None]
    wv_f = np.asarray(inputs["wv"], np.float32) * g1[:, :, None]
    wq_f = _deinterleave(wq_f) * (1.0 / np.sqrt(DK, dtype=np.float32))
    wk_f = _deinterleave(wk_f)
    bq_f = _deinterleave(np.asarray(inputs["bq"], np.float32)) / np.sqrt(DK)
    bk_f = _deinterleave(np.asarray(inputs["bk"], np.float32))
    bv_f = np.asarray(inputs["bv"], np.float32)
    wo_f = np.asarray(inputs["wo"], np.float32)
    bo_f = np.asarray(inputs["bo"], np.float32)
    w1_f = np.asarray(inputs["w1"], np.float32) * g2[:, :, None]
    w2_f = np.asarray(inputs["w2"], np.float32) * g2[:, :, None]
    w3_f = np.asarray(inputs["w3"], np.float32)
    b1_f = np.asarray(inputs["b1"], np.float32)
    b2_f = np.asarray(inputs["b2"], np.float32)
    b3_f = np.asarray(inputs["b3"], np.float32)

    # shared tensors
    w1p = np.zeros((L, D, FDP), np.float32); w1p[:, :, :FD] = w1_f
    w2p = np.zeros((L, D, FDP), np.float32); w2p[:, :, :FD] = w2_f
    w3p = np.zeros((L, FDP, D), np.float32); w3p[:, :FD, :] = w3_f
    b1p = np.zeros((L, FDP), np.float32); b1p[:, :FD] = b1_f
    b2p = np.zeros((L, FDP), np.float32); b2p[:, :FD] = b2_f

    # pre-tiled shared weights
    wo_t = np.ascontiguousarray(
        wo_f.reshape(L, DSUB, 128, DSUB, 128)      # l, kt, p, m, c
            .transpose(0, 3, 2, 1, 4))             # l, m, p, kt, c
    w1_t = np.ascontiguousarray(
        w1p.reshape(L, DSUB, 128, FM, 128)         # l, o, p, m, c
           .transpose(0, 3, 2, 1, 4))              # l, m, p, o, c
    w2_t = np.ascontiguousarray(
        w2p.reshape(L, DSUB, 128, FM, 128).transpose(0, 3, 2, 1, 4))
    w3_t = np.ascontiguousarray(
        w3p.reshape(L, FM, 128, DSUB, 128)         # l, ko, p, m, c
           .transpose(0, 3, 2, 1, 4))              # l, m, p, ko, c
    bo_t = np.ascontiguousarray(bo_f.reshape(L, DSUB, 128).transpose(0, 2, 1))
    b1_t = np.ascontiguousarray(b1p.reshape(L, FM, 128).transpose(0, 2, 1))
    b2_t = np.ascontiguousarray(b2p.reshape(L, FM, 128).transpose(0, 2, 1))
    b3_t = np.ascontiguousarray(b3_f.reshape(L, DSUB, 128).transpose(0, 2, 1))

    # rope tables
    c1_t = np.empty((DK, TB), np.float32)
    c2_t = np.empty((DK, TB), np.float32)
    c1_t[0:32] = fcos.T; c1_t[32:64] = fcos.T
    c2_t[0:32] = -fsin.T; c2_t[32:64] = fsin.T
    psw = np.zeros((DK, DK), np.float32)
    for i in range(32):
        psw[i, i + 32] = 1.0
        psw[i + 32, i] = 1.0

    embT = embed.T * gpost[:, None]                # [D, V]

    x_emb = embed[tokens]                          # [B, S, D]

    in_maps = []
    for c in range(N_CORES):
        # x0T: feature-major [128, DSUB, T] for this core's tokens
        xc = x_emb[:, TC * c:TC * (c + 1), :]      # [B, TC, D]
        x0T = np.ascontiguousarray(
            xc.reshape(B * TC, D).T.reshape(DSUB, 128, T).transpose(1, 0, 2))
        wq_c = np.ascontiguousarray(
            wq_f[:, :, 128 * c:128 * (c + 1)]
            .reshape(L, DSUB, 128, 128).transpose(0, 2, 1, 3))   # l p o m
        wk_c = np.ascontiguousarray(
            wk_f[:, :, DK * c:DK * (c + 1)]
            .reshape(L, DSUB, 128, DK).transpose(0, 2, 1, 3))
        wv_c = np.ascontiguousarray(
            wv_f[:, :, DK * c:DK * (c + 1)]
            .reshape(L, DSUB, 128, DK).transpose(0, 2, 1, 3))
        embH_c = np.zeros((D, VSHP), np.float32)
        embH_c[:, :VSH] = embT[:, VSH * c:VSH * (c + 1)]
        embH_t = np.ascontiguousarray(
            embH_c.reshape(DSUB, 128, VSHP).transpose(1, 0, 2))
        bvB_c = np.broadcast_to(bv_f[:, None, DK * c:DK * (c + 1)],
                                (L, 128, DK)).copy()
        in_maps.append({
            "x0T": x0T.astype(np.float32),
            "wq": wq_c.astype(BF), "wk": wk_c.astype(BF), "wv": wv_c.astype(BF),
            "wo": wo_t.astype(BF), "w1": w1_t.astype(BF), "w2": w2_t.astype(BF),
            "w3": w3_t.astype(BF), "embH": embH_t.astype(BF),
            "c1": c1_t, "c2": c2_t, "pswap": psw.astype(BF),
            "ident": np.eye(DK, dtype=np.float32).astype(BF),
            "bq": np.ascontiguousarray(bq_f[:, 128 * c:128 * (c + 1)]).astype(np.float32),
            "bk": np.ascontiguousarray(bk_f[:, DK * c:DK * (c + 1)]).astype(np.float32),
            "bvB": bvB_c.astype(np.float32), "bo": bo_t, "b1": b1_t,
            "b2": b2_t, "b3": b3_t,
        })
    return in_maps


# ---------------------------------------------------------------- runner

def _make_runner(nc):
    """Cached sharded executable (mirrors bass2jax.run_bass_via_pjrt)."""
    import jax
    from jax.sharding import Mesh, PartitionSpec
    from jax.experimental.shard_map import shard_map

    bass2jax.install_neuronx_cc_hook()
    in_names, out_names, out_avals, zero_shapes = [], [], [], []
    pid_name = nc.partition_id_tensor.name if nc.partition_id_tensor else None
    for alloc in nc.m.functions[0].allocations:
        if not isinstance(alloc, mybir.MemoryLocationSet):
            continue
        name = alloc.memorylocations[0].name
        if alloc.kind == "ExternalInput":
            if name != pid_name:
                in_names.append(name)
        elif alloc.kind == "ExternalOutput":
            out_names.append(name)
            shape = tuple(alloc.tensor_shape)
            dtype = mybir.dt.np(alloc.dtype)
            out_avals.append(jax.core.ShapedArray(shape, dtype))
            zero_shapes.append((shape, dtype))
    n_params = len(in_names)
    all_names = list(in_names) + list(out_names)
    if pid_name is not None:
        all_names.append(pid_name)

    def _body(*args):
        operands = list(args)
        if pid_name is not None:
            operands.append(bass2jax.partition_id_tensor())
        outs = bass2jax._bass_exec_p.bind(
            *operands, out_avals=tuple(out_avals), in_names=tuple(all_names),
            out_names=tuple(out_names), lowering_input_output_aliases=(),
            sim_require_finite=True, sim_require_nnan=True, nc=nc)
        return tuple(outs)

    devices = jax.devices()[:N_CORES]
    mesh = Mesh(np.asarray(devices), ("core",))
    _CACHE["mesh"] = mesh
    n_outs = len(out_names)
    sharded = jax.jit(
        shard_map(_body, mesh=mesh,
                  in_specs=(PartitionSpec("core"),) * (n_params + n_outs),
                  out_specs=(PartitionSpec("core"),) * n_outs,
                  check_rep=False),
        donate_argnums=tuple(range(n_params, n_params + n_outs)),
        keep_unused=True)
    return sharded, in_names, out_names, zero_shapes


def _get_exec():
    if "exec" not in _CACHE:
        nc = build_program()
        _CACHE["exec"] = _make_runner(nc)
    return _CACHE["exec"]


def commit_inputs(in_maps):
    """Concat per-core inputs and upload to the device mesh once."""
    import jax
    from jax.sharding import NamedSharding, PartitionSpec
    sharded, in_names, out_names, zero_shapes = _get_exec()
    mesh = _CACHE["mesh"]
    sh = NamedSharding(mesh, PartitionSpec("core"))
    dev_in = []
    for n in in_names:
        arr = np.concatenate([m[n] for m in in_maps], axis=0)
        dev_in.append(jax.device_put(arr, sh))
    jax.block_until_ready(dev_in)
    return dev_in


def _zeros_fn():
    import jax
    import jax.numpy as jnp
    from jax.sharding import NamedSharding, PartitionSpec
    if "zeros_fn" not in _CACHE:
        sharded, in_names, out_names, zero_shapes = _get_exec()
        mesh = _CACHE["mesh"]
        sh = NamedSharding(mesh, PartitionSpec("core"))
        def mk():
            return tuple(jnp.zeros((N_CORES * s[0], *s[1:]), d)
                         for s, d in zero_shapes)
        _CACHE["zeros_fn"] = jax.jit(mk, out_shardings=(sh,) * len(zero_shapes))
    return _CACHE["zeros_fn"]


def run_device_committed(dev_in, as_numpy=True):
    """Run with pre-committed device inputs; returns raw jax outputs or numpy."""
    import jax
    sharded, in_names, out_names, zero_shapes = _get_exec()
    zeros = _zeros_fn()()
    outs = sharded(*dev_in, *zeros)
    jax.block_until_ready(outs)
    if not as_numpy:
        return outs
    outs = [np.asarray(o) for o in outs]
    res = []
    for c in range(N_CORES):
        res.append({n: outs[i].reshape(N_CORES, *zero_shapes[i][0])[c]
                    for i, n in enumerate(out_names)})
    return res


def run_device(in_maps):
    return run_device_committed(commit_inputs(in_maps))


# ---------------------------------------------------------------- entry point

def kernel(**inputs):
    in_maps = _prep_inputs(inputs)
    res = run_device(in_maps)
    # assemble [B, S, V] from per-core vocab shards of all tokens
    logits = np.empty((B, S, V), np.float32)
    for c in range(N_CORES):
        lc = res[c]["logits"]                       # [4096, VSHP]
        lc = lc[:, :VSH].reshape(N_CORES, B, TC, VSH)
        for r in range(N_CORES):
            logits[:, TC * r:TC * (r + 1), VSH * c:VSH * (c + 1)] = lc[r]
    return logits
